# revision 3
# baseline (speedup 1.0000x reference)
"""BitLinear forward (ternary-quantized linear) on 8 Trainium2 NeuronCores.

Computes out = x @ (clip(round(w/0.5), -1, 1) * scale[:, None]).T
for x:[4,2048,4096] f32, w:[11008,4096] f32, scale:[11008] f32.

Strategy (column-parallel, per the spec sharding hint):
  - Shard weight/scale along out_f: core c gets rows [c*1376, (c+1)*1376).
  - Replicate x; each core computes out[:, c*1376:(c+1)*1376].
  - Host passes x and the weight shard TRANSPOSED (contraction dim in_f
    outermost) so every device DMA is a natural-layout load; the gather is
    a concatenate along the feature axis.

Device kernel (per core):
  - DMA wT shard f32, quantize on device to ternary*scale, cached in SBUF
    as fp16 (ternary values are exact in fp16; x is the only rounded input).
  - Stream x m-tiles (128 tokens), cast f32->fp16 on DVE.
  - PE: out-tile [128 tok x {512,512,352} outf] accumulated over 32 k-tiles
    in PSUM (fp32); fp16 matmul runs at 1 cycle/row (4x faster than fp32).
  - ACT copies PSUM->SBUF, DMA to DRAM.
"""

import os

import numpy as np

import concourse.bass as bass
import concourse.mybir as mybir
import concourse.tile as tile
from concourse import bacc
from concourse.bass_utils import run_bass_kernel_spmd

P = 128
IN_F = 4096
OUT_F = 11008
BATCH = 4
SEQ = 2048
TOKENS = BATCH * SEQ  # 8192
N_CORES = 8
NSH = OUT_F // N_CORES  # 1376 out features per core

MAGIC = None  # unused; quantization is sign(w) * (|w| > 0.25)


def _n_chunks(nsh):
    """Split the out_f shard into moving-operand chunks of <=512 (PSUM bank)."""
    chunks = []
    n0 = 0
    while n0 < nsh:
        nw = min(512, nsh - n0)
        chunks.append((n0, nw))
        n0 += nw
    return chunks


def build_program(in_f=IN_F, tokens=TOKENS, nsh=NSH):
    """Build + compile the per-core Bass program (same program on all cores)."""
    ko_n = in_f // P  # k-tiles
    mt_n = tokens // P  # m-tiles (token tiles)
    chunks = _n_chunks(nsh)
    # x f32 staging granularity: ko-quarters keep SBUF pressure low
    stage_ko = min(8, ko_n)

    nc = bacc.Bacc("TRN2", target_bir_lowering=False, debug=False)

    xT = nc.dram_tensor("xT", [in_f, tokens], mybir.dt.float32, kind="ExternalInput")
    wT = nc.dram_tensor("wT", [in_f, nsh], mybir.dt.float32, kind="ExternalInput")
    scale = nc.dram_tensor("scale", [nsh], mybir.dt.float32, kind="ExternalInput")
    out = nc.dram_tensor("out", [tokens, nsh], mybir.dt.float32, kind="ExternalOutput")

    xT_ap = xT.ap().rearrange("(ko p) t -> p ko t", p=P)  # [128, ko_n, tokens]
    wT_ap = wT.ap()
    out_ap = out.ap()

    f32 = mybir.dt.float32
    f16 = mybir.dt.float16
    Alu = mybir.AluOpType

    with tile.TileContext(nc) as tc:
        with (
            tc.tile_pool(name="const", bufs=1) as const,
            tc.tile_pool(name="wqp", bufs=1) as wqp,
            tc.tile_pool(name="wst", bufs=2) as wst_pool,
            tc.tile_pool(name="qtmp", bufs=2) as qtmp,
            tc.tile_pool(name="xst", bufs=2) as xst_pool,
            tc.tile_pool(name="x16", bufs=2) as x16_pool,
            tc.tile_pool(name="ot", bufs=2) as ot_pool,
            tc.tile_pool(name="psum", bufs=6, space="PSUM") as psum,
        ):
            # scale broadcast across partitions: [128, nsh]
            scale_bc = const.tile([P, nsh], f32)
            sc_ap = scale.ap()
            sc_bcast = bass.AP(
                tensor=sc_ap.tensor, offset=sc_ap.offset, ap=[[0, P], *sc_ap.ap]
            )
            nc.sync.dma_start(scale_bc, sc_bcast)

            # Quantize the full weight shard once, cache as fp16 [128, ko, nsh]
            wq = wqp.tile([P, ko_n, nsh], f16)
            for ko in range(ko_n):
                wst = wst_pool.tile([P, nsh], f32, tag="wst")
                nc.sync.dma_start(wst, wT_ap[ko * P : (ko + 1) * P, :])
                # ternary quant == (w > 0.25) - (w < -0.25); boundary values
                # land on round-half-even zero exactly like jnp.round(w/0.5)
                pos = qtmp.tile([P, nsh], f32, tag="pos")
                nc.vector.tensor_scalar(pos, wst, 0.25, None, Alu.is_gt)
                neg = qtmp.tile([P, nsh], f32, tag="neg")
                nc.vector.tensor_scalar(neg, wst, -0.25, None, Alu.is_lt)
                tern = qtmp.tile([P, nsh], f32, tag="pos")
                nc.vector.tensor_tensor(tern, pos, neg, Alu.subtract)
                nc.vector.tensor_tensor(wq[:, ko, :], tern, scale_bc, Alu.mult)

            # Main matmul loop over token tiles
            for mt in range(mt_n):
                m0 = mt * P
                x16 = x16_pool.tile([P, ko_n, P], f16)
                for q0 in range(0, ko_n, stage_ko):
                    xst = xst_pool.tile([P, stage_ko, P], f32, tag="xst")
                    nc.sync.dma_start(xst, xT_ap[:, q0 : q0 + stage_ko, m0 : m0 + P])
                    nc.vector.tensor_copy(x16[:, q0 : q0 + stage_ko, :], xst)

                ot = ot_pool.tile([P, nsh], f32)
                for n0, nw in chunks:
                    ps = psum.tile([P, 512], f32, tag="ps")
                    for ko in range(ko_n):
                        nc.tensor.matmul(
                            ps[:, :nw],
                            x16[:, ko, :],
                            wq[:, ko, n0 : n0 + nw],
                            start=(ko == 0),
                            stop=(ko == ko_n - 1),
                        )
                    nc.scalar.copy(ot[:, n0 : n0 + nw], ps[:, :nw])
                nc.sync.dma_start(out_ap[m0 : m0 + P, :], ot)

    nc.compile()
    return nc


_PROGRAM = None


def _get_program():
    global _PROGRAM
    if _PROGRAM is None:
        _PROGRAM = build_program()
    return _PROGRAM


def _patch_artifact_upload():
    """Tracing uploads the NEFF dir to a shared bucket; in this container that
    can fail (no credentials) - degrade to a local-path no-op."""
    import concourse.bass_utils as bu

    orig = bu.upload_artifacts

    def safe_upload(tmpdir):
        try:
            return orig(tmpdir)
        except Exception:
            return tmpdir

    bu.upload_artifacts = safe_upload


def kernel(x, weight, scale):
    x = np.asarray(x, dtype=np.float32)
    weight = np.asarray(weight, dtype=np.float32)
    scale = np.asarray(scale, dtype=np.float32)

    xT = np.ascontiguousarray(x.reshape(TOKENS, IN_F).T)  # [in_f, tokens]
    in_maps = []
    for c in range(N_CORES):
        wc = weight[c * NSH : (c + 1) * NSH]  # [nsh, in_f]
        in_maps.append(
            {
                "xT": xT,
                "wT": np.ascontiguousarray(wc.T),  # [in_f, nsh]
                "scale": np.ascontiguousarray(scale[c * NSH : (c + 1) * NSH]),
            }
        )

    nc = _get_program()
    trace = os.environ.get("BASS_TRACE", "") == "1"
    if trace:
        _patch_artifact_upload()
    res = run_bass_kernel_spmd(nc, in_maps, core_ids=list(range(N_CORES)), trace=trace)
    kernel.last_results = res

    out = np.concatenate([res.results[c]["out"] for c in range(N_CORES)], axis=1)
    return out.reshape(BATCH, SEQ, OUT_F)


kernel.last_results = None
